# revision 1
# baseline (speedup 1.0000x reference)
"""Bass/Trainium2 kernel for nn_GatherUpdate: LayerNorm + Linear + per-atom
row gather + residual add, data-parallel over batch across 8 NeuronCores.

reference:
    normed = LayerNorm(s) * gamma + beta            # s: [B, 2048, 384]
    upd    = normed @ W.T                           # W: [128, 384] -> [B, 2048, 128]
    out    = atom_embed + upd[:, cond_to_s_idx, :]  # atom_embed: [B, 32768, 128]

Per-core plan (core b handles batch b):
  1. LN over 16 tiles of [128 res, 384] via bn_stats/bn_aggr.
  2. PE-transpose normed tiles (cs onto partitions), matmul against
     gamma-folded W^T chunks, accumulate in PSUM; beta folded in via a K=1
     ones-row matmul against (beta @ W.T).
  3. Write upd table [2048, 128] to DRAM scratch.
  4. dma_gather 512B rows from the table (32 calls x 1024 idxs — the HW
     SWDGE ring holds ~127 descriptors/engine — rotated over 4 SWDGE
     queues), add to atom_embed tiles, store. Atom tiles use contiguous
     4MB DMAs; the host pre-permutes the int16 index stream so the gather
     output layout matches the atom tiles.
"""

import sys

sys.path.insert(0, "/opt/trn_rl_repo")

import numpy as np

B = 8
N_ATOMS = 32768
N_RES = 2048
C_S = 384
C_ATOM = 128
EPS = 1e-5
P = 128
N_RES_TILES = N_RES // P  # 16
N_CHUNKS = 4  # atom chunks per core
CHUNK = N_ATOMS // N_CHUNKS  # 8192 atoms
CT = CHUNK // P  # 64 tiles of 128 atoms per chunk
KC = C_S // P  # 3 contraction chunks
GSUB = 1024  # idxs per dma_gather (HW SWDGE ring holds ~127 desc/engine)
GQ = CHUNK // GSUB  # 8 sub-gathers per atom chunk
GT = GSUB // P  # 8 x 128-atom tiles per sub-gather
NQUEUES = 4  # rotate SWDGE queues so desc-gen overlaps in-flight gathers

_compiled = None


def _build(repeat=1):
    """Build the per-core program. repeat>1 unrolls the whole pipeline N
    times (used only for timing: wall(N)-wall(1) cancels dispatch/transfer
    overhead)."""
    import concourse.bacc as bacc
    import concourse.tile as tile
    from concourse import mybir
    from concourse.masks import make_identity
    from concourse.tile import add_dep_helper

    f32 = mybir.dt.float32

    nc = bacc.Bacc(
        "TRN2", target_bir_lowering=False, debug=False, num_swdge_queues=NQUEUES
    )

    atom = nc.dram_tensor("atom", [N_ATOMS, C_ATOM], f32, kind="ExternalInput")
    s_in = nc.dram_tensor("s_in", [N_RES, C_S], f32, kind="ExternalInput")
    idx16 = nc.dram_tensor(
        "idx16", [P, N_ATOMS // 16], mybir.dt.int16, kind="ExternalInput"
    )
    wg = nc.dram_tensor("wg", [P, C_S], f32, kind="ExternalInput")
    wt = nc.dram_tensor("wt", [P, C_S], f32, kind="ExternalInput")
    beta3 = nc.dram_tensor("beta3", [P, KC], f32, kind="ExternalInput")
    out = nc.dram_tensor("out", [N_ATOMS, C_ATOM], f32, kind="ExternalOutput")
    upd_dram = nc.dram_tensor("upd_scratch", [N_RES, C_ATOM], f32, kind="Internal")

    with tile.TileContext(nc) as tc:
        with (
            tc.tile_pool(name="singles", bufs=1) as singles,
            tc.tile_pool(name="stats", bufs=4) as stats,
            tc.tile_pool(name="t2p", bufs=3) as t2p,
            tc.tile_pool(name="t2tp", bufs=6) as t2tp,
            tc.tile_pool(name="psum_tt", bufs=4, space="PSUM") as psum_tt,
            tc.tile_pool(name="psum_mm", bufs=2, space="PSUM") as psum_mm,
            tc.tile_pool(name="atoms", bufs=2) as atoms_pool,
            tc.tile_pool(name="gath", bufs=2) as gath_pool,
        ):
            # --- s load first: it gates LN start, so its first quarter must
            # win the DMA engines before the constants below. p-major:
            # s_big[p, j, :] = row p*16 + j -> contiguous per partition
            # (LN is per-row, so any row->lane mapping works).
            s_pt = s_in.ap().rearrange("(p t) c -> p t c", p=P)

            def load_s():
                sb = singles.tile([P, N_RES_TILES, C_S], f32, tag="s_big")
                for sq in range(4):
                    tq = N_RES_TILES // 4
                    nc.sync.dma_start(
                        out=sb[:, sq * tq : (sq + 1) * tq, :],
                        in_=s_pt[:, sq * tq : (sq + 1) * tq, :],
                    )
                return sb

            s_first = load_s()

            # --- constants / small inputs (idx last: gathers need it ~40us in)
            wg_sb = singles.tile([P, C_S], f32)
            nc.sync.dma_start(out=wg_sb[:], in_=wg.ap())
            wt_sb = singles.tile([P, C_S], f32)
            nc.sync.dma_start(out=wt_sb[:], in_=wt.ap())
            beta_sb = singles.tile([P, KC], f32)
            nc.sync.dma_start(out=beta_sb[:], in_=beta3.ap())
            idx_sb = singles.tile([P, N_ATOMS // 16], mybir.dt.int16)
            nc.sync.dma_start(out=idx_sb[:], in_=idx16.ap())

            ident = singles.tile([P, P], f32)
            make_identity(nc, ident[:])
            ones1 = singles.tile([1, P], f32)
            nc.vector.memset(ones1[:], 1.0)
            eps_t = singles.tile([P, 1], f32)
            nc.vector.memset(eps_t[:], EPS)

            # --- bias row: (beta @ W.T) [1, C_ATOM] ---
            bias_ps = psum_mm.tile([1, C_ATOM], f32, tag="biasps")
            for k in range(KC):
                nc.tensor.matmul(
                    bias_ps[:],
                    lhsT=beta_sb[:, k : k + 1],
                    rhs=wt_sb[:, k * P : (k + 1) * P],
                    start=(k == 0),
                    stop=(k == KC - 1),
                )
            bias_sb = singles.tile([1, C_ATOM], f32)
            nc.vector.tensor_copy(out=bias_sb[:], in_=bias_ps[:])
            # broadcast bias across partitions once: ones[1,P].T @ bias[1,P]
            bias_bc_ps = psum_mm.tile([P, C_ATOM], f32, tag="updps")
            nc.tensor.matmul(
                bias_bc_ps[:], lhsT=ones1[:], rhs=bias_sb[:], start=True, stop=True
            )
            bias_bc = singles.tile([P, C_ATOM], f32)
            nc.vector.tensor_copy(out=bias_bc[:], in_=bias_bc_ps[:])

            prev_gathers = []
            for _rep in range(repeat):
                s_big = s_first if _rep == 0 else load_s()

                # --- LN + matmul into upd table ---
                upd_big = singles.tile([P, N_RES_TILES, C_ATOM], f32, tag="upd_big")
                for i in range(N_RES_TILES):
                    st6 = stats.tile([P, 6], f32)
                    nc.vector.bn_stats(out=st6[:], in_=s_big[:, i, :])
                    mv = stats.tile([P, 2], f32)
                    nc.vector.bn_aggr(out=mv[:], in_=st6[:])
                    std = stats.tile([P, 1], f32)
                    nc.scalar.activation(
                        out=std[:],
                        in_=mv[:, 1:2],
                        func=mybir.ActivationFunctionType.Sqrt,
                        bias=eps_t[:],
                    )
                    rstd = stats.tile([P, 1], f32)
                    nc.vector.reciprocal(out=rstd[:], in_=std[:])
                    t2 = t2p.tile([P, C_S], f32)
                    nc.vector.tensor_scalar(
                        out=t2[:],
                        in0=s_big[:, i, :],
                        scalar1=mv[:, 0:1],
                        scalar2=rstd[:],
                        op0=mybir.AluOpType.subtract,
                        op1=mybir.AluOpType.mult,
                    )
                    t2t_chunks = []
                    for k in range(KC):
                        tt_ps = psum_tt.tile([P, P], f32)
                        nc.tensor.transpose(
                            out=tt_ps[:],
                            in_=t2[:, k * P : (k + 1) * P],
                            identity=ident[:],
                        )
                        t2t = t2tp.tile([P, P], f32)
                        nc.vector.tensor_copy(out=t2t[:], in_=tt_ps[:])
                        t2t_chunks.append(t2t)
                    upd_ps = psum_mm.tile([P, C_ATOM], f32, tag="updps")
                    for k in range(KC):
                        nc.tensor.matmul(
                            upd_ps[:],
                            lhsT=t2t_chunks[k][:],
                            rhs=wg_sb[:, k * P : (k + 1) * P],
                            start=(k == 0),
                            stop=(k == KC - 1),
                        )
                    # fold the beta bias in during the PSUM->SBUF move (DVE)
                    # instead of a 4th K=1 matmul: keeps PE on the critical
                    # path free for transposes/matmuls
                    nc.vector.tensor_tensor(
                        out=upd_big[:, i, :],
                        in0=upd_ps[:],
                        in1=bias_bc[:],
                        op=mybir.AluOpType.add,
                    )

                # store table in quarters so stores overlap later LN tiles;
                # gathers gate on all four. p-major: upd_big[p, j, :] is
                # table row p*16 + j -> contiguous per partition.
                upd_stores = []
                upd_pt = upd_dram.ap().rearrange("(p t) c -> p t c", p=P)
                for sq in range(4):
                    tq = N_RES_TILES // 4
                    us = nc.sync.dma_start(
                        out=upd_pt[:, sq * tq : (sq + 1) * tq, :],
                        in_=upd_big[:, sq * tq : (sq + 1) * tq, :],
                    )
                    upd_stores.append(us)
                # WAR: don't overwrite the table while last rep still gathers
                for pg in prev_gathers:
                    for us in upd_stores:
                        add_dep_helper(
                            us.ins, pg.ins, reason="WAR on upd table across reps"
                        )
                prev_gathers = []

                # --- gather + residual add over 4 chunks of 8192 atoms ---
                for c in range(N_CHUNKS):
                    at = atoms_pool.tile([P, CT, C_ATOM], f32, tag="at")
                    nc.sync.dma_start(
                        out=at[:],
                        in_=atom.ap()[c * CHUNK : (c + 1) * CHUNK, :].rearrange(
                            "(p t) c -> p t c", p=P
                        ),
                    )
                    g = gath_pool.tile([P, CT, C_ATOM], f32, tag="g")
                    for q in range(GQ):
                        gi = nc.gpsimd.dma_gather(
                            g[:, q * GT : (q + 1) * GT, :],
                            upd_dram.ap(),
                            idx_sb[
                                :,
                                c * (CHUNK // 16) + q * (GSUB // 16) : c * (CHUNK // 16)
                                + (q + 1) * (GSUB // 16),
                            ],
                            GSUB,
                            GSUB,
                            C_ATOM,
                            queue_num=(c * GQ + q) % NQUEUES,
                        )
                        for us in upd_stores:
                            add_dep_helper(
                                gi.ins, us.ins, reason="upd table must be in DRAM"
                            )
                        prev_gathers.append(gi)
                    nc.vector.tensor_add(out=at[:], in0=at[:], in1=g[:])
                    nc.sync.dma_start(
                        out=out.ap()[c * CHUNK : (c + 1) * CHUNK, :].rearrange(
                            "(p t) c -> p t c", p=P
                        ),
                        in_=at[:],
                    )

    nc.compile()
    return nc


def _prep_core_inputs(atom_embed, s, cond_to_s_idx, ln_gamma, ln_beta, W):
    """Host-side sharding + layout marshalling (no math beyond folding the
    LN scale into the weight layout)."""
    # gamma-folded W^T, chunked so cs-chunk k sits at free columns [k*128, ...)
    wg_full = (W * ln_gamma[None, :]).T.astype(np.float32)  # [C_S, C_ATOM]
    wg_host = np.ascontiguousarray(
        wg_full.reshape(KC, P, C_ATOM).transpose(1, 0, 2).reshape(P, C_S)
    )
    wt_full = np.ascontiguousarray(W.T.astype(np.float32))  # [C_S, C_ATOM]
    wt_host = np.ascontiguousarray(
        wt_full.reshape(KC, P, C_ATOM).transpose(1, 0, 2).reshape(P, C_S)
    )
    beta_host = np.ascontiguousarray(
        ln_beta.astype(np.float32).reshape(KC, P).T
    )  # [P, KC]

    in_maps = []
    for b in range(B):
        idxb = cond_to_s_idx[b].astype(np.int16)  # values < 2048
        # atom tile layout within a 4096-chunk: at[p, t] = atom p*CT + t.
        # sub-gather q writes g[j%128, q*GT + j//128] for list position j, so
        # position j of sub-gather q must hold the index of atom
        # (j%128)*CT + q*GT + j//128.
        A = idxb.reshape(N_CHUNKS, P, GQ, GT)  # [c, p, q, u]
        L = A.transpose(0, 2, 3, 1).reshape(N_CHUNKS, GQ, GSUB)  # j = u*128+p
        # wrap within each sub-gather: position j -> [j%16, j//16]
        Wr = L.reshape(N_CHUNKS, GQ, GSUB // 16, 16).transpose(0, 1, 3, 2)
        idx_full = np.ascontiguousarray(
            Wr.transpose(2, 0, 1, 3).reshape(16, N_ATOMS // 16)
        )
        idx_rep = np.ascontiguousarray(np.tile(idx_full, (P // 16, 1)))
        in_maps.append(
            {
                "atom": np.ascontiguousarray(atom_embed[b]),
                "s_in": np.ascontiguousarray(s[b]),
                "idx16": idx_rep,
                "wg": wg_host,
                "wt": wt_host,
                "beta3": beta_host,
            }
        )
    return in_maps


def kernel(atom_embed, s, cond_to_s_idx, ln_gamma, ln_beta, W):
    global _compiled
    from concourse.bass_utils import run_bass_kernel_spmd

    atom_embed = np.asarray(atom_embed, dtype=np.float32)
    s = np.asarray(s, dtype=np.float32)
    cond_to_s_idx = np.asarray(cond_to_s_idx)
    ln_gamma = np.asarray(ln_gamma, dtype=np.float32)
    ln_beta = np.asarray(ln_beta, dtype=np.float32)
    W = np.asarray(W, dtype=np.float32)

    if _compiled is None:
        _compiled = _build()
    in_maps = _prep_core_inputs(atom_embed, s, cond_to_s_idx, ln_gamma, ln_beta, W)
    res = run_bass_kernel_spmd(_compiled, in_maps, core_ids=list(range(B)))
    out = np.stack([res.results[b]["out"] for b in range(B)], axis=0)
    return out



# revision 3
# speedup vs baseline: 135.3489x; 135.3489x over previous
"""Bass/Trainium2 kernel for nn_GatherUpdate: LayerNorm + Linear + per-atom
row gather + residual add, data-parallel over batch across 8 NeuronCores.

reference:
    normed = LayerNorm(s) * gamma + beta            # s: [B, 2048, 384]
    upd    = normed @ W.T                           # W: [128, 384] -> [B, 2048, 128]
    out    = atom_embed + upd[:, cond_to_s_idx, :]  # atom_embed: [B, 32768, 128]

Per-core plan (core b handles batch b):
  1. LN over 16 tiles of [128 res, 384] via bn_stats/bn_aggr (s held in bf16:
     tolerance is 2e-2, bf16 error ~4e-3, and it halves DMA traffic).
  2. PE-transpose normed tiles, then matmul with lhsT = gamma-folded W^T so
     PSUM directly receives the TRANSPOSED update table updT[c_atom, res].
     Bias (beta @ W.T) folded in during the PSUM->SBUF move as a per-partition
     scalar add. The 2048x128 table lives in SBUF only - no DRAM roundtrip.
  3. Gather with gpsimd ap_gather (SBUF->SBUF along the free dim, no DMA
     descriptors): g[c, j] = updT[c, idx[j]]. Host pre-remaps index values to
     table columns and pre-permutes the list so downstream tiles line up.
  4. Per 128-atom tile: PE transpose g columns back to [atom, c] into PSUM
     (start=True), then accumulate the atom_embed tile on top with an
     identity-matmul (start=False). ACT engine copies PSUM->SBUF (bf16),
     batched 4 tiles per PSUM bank. DMA out. atom_embed in/out move as bf16
     with 32KB-contiguous per-partition lines.
"""

import sys

sys.path.insert(0, "/opt/trn_rl_repo")

import numpy as np
import ml_dtypes

B = 8
N_ATOMS = 32768
N_RES = 2048
C_S = 384
C_ATOM = 128
EPS = 1e-5
P = 128
NT = N_RES // P  # 16 res tiles
KC = C_S // P  # 3 contraction chunks
G = 4096  # atoms per gather/DMA chunk
NG = N_ATOMS // G  # 8 chunks
CT = G // P  # 32 tiles of 128 atoms per chunk
GRP = 4  # PE-transposed tiles batched per PSUM bank copy

_compiled = None


def _build(repeat=1, hw_loop=None):
    """Build the per-core program. repeat>1 python-unrolls the body N times;
    hw_loop=N instead wraps the body in a device-side For_i hardware loop
    (used for low-noise timing: signal scales with N at constant program
    size)."""
    import concourse.bacc as bacc
    import concourse.tile as tile
    from concourse import mybir
    from concourse.masks import make_identity

    f32 = mybir.dt.float32
    bf16 = mybir.dt.bfloat16
    i16 = mybir.dt.int16

    nc = bacc.Bacc("TRN2", target_bir_lowering=False, debug=False)

    atom = nc.dram_tensor("atom", [N_ATOMS, C_ATOM], bf16, kind="ExternalInput")
    s_in = nc.dram_tensor("s_in", [N_RES, C_S], bf16, kind="ExternalInput")
    idx16 = nc.dram_tensor("idx16", [P, N_ATOMS // 16], i16, kind="ExternalInput")
    wg = nc.dram_tensor("wg", [P, C_S], bf16, kind="ExternalInput")
    wt = nc.dram_tensor("wt", [P, C_S], f32, kind="ExternalInput")
    beta3 = nc.dram_tensor("beta3", [P, KC], f32, kind="ExternalInput")
    out = nc.dram_tensor("out", [N_ATOMS, C_ATOM], bf16, kind="ExternalOutput")

    with tile.TileContext(nc) as tc:
        with (
            tc.tile_pool(name="singles", bufs=1) as singles,
            tc.tile_pool(name="spool", bufs=2) as spool,
            tc.tile_pool(name="updp", bufs=2) as updp,
            tc.tile_pool(name="stats", bufs=4) as stats,
            tc.tile_pool(name="t2p", bufs=3) as t2p,
            tc.tile_pool(name="t2tp", bufs=6) as t2tp,
            tc.tile_pool(name="psum_tt", bufs=2, space="PSUM") as psum_tt,
            tc.tile_pool(name="psum_mm", bufs=2, space="PSUM") as psum_mm,
            tc.tile_pool(name="psum_g", bufs=4, space="PSUM") as psum_g,
            tc.tile_pool(name="atoms", bufs=3) as atoms_pool,
            tc.tile_pool(name="gath", bufs=2) as gath_pool,
            tc.tile_pool(name="outp", bufs=2) as out_pool,
        ):
            # --- constants / small inputs (loaded once) ---
            idx_sb = singles.tile([P, N_ATOMS // 16], i16)
            nc.sync.dma_start(out=idx_sb[:], in_=idx16.ap())
            wg_sb = singles.tile([P, C_S], bf16)
            nc.sync.dma_start(out=wg_sb[:], in_=wg.ap())
            wt_sb = singles.tile([P, C_S], f32)
            nc.sync.dma_start(out=wt_sb[:], in_=wt.ap())
            beta_sb = singles.tile([P, KC], f32)
            nc.sync.dma_start(out=beta_sb[:], in_=beta3.ap())

            ident_f = singles.tile([P, P], f32)
            make_identity(nc, ident_f[:])
            ident_b = singles.tile([P, P], bf16)
            make_identity(nc, ident_b[:])
            eps_t = singles.tile([P, 1], f32)
            nc.vector.memset(eps_t[:], EPS)

            # --- bias column: (beta @ W.T) as [C_ATOM, 1] ---
            # (borrows an updps-tag bank; PSUM has no room for a its own tag)
            bias_ps = psum_mm.tile([P, P], f32, tag="updps")
            for k in range(KC):
                nc.tensor.matmul(
                    bias_ps[:, 0:1],
                    lhsT=wt_sb[:, k * P : (k + 1) * P],
                    rhs=beta_sb[:, k : k + 1],
                    start=(k == 0),
                    stop=(k == KC - 1),
                )
            bias_sb = singles.tile([P, 1], f32)
            nc.vector.tensor_copy(out=bias_sb[:], in_=bias_ps[:, 0:1])

            s_pt = s_in.ap().rearrange("(p t) c -> p t c", p=P)

            def body():
                # --- s load (quarters so LN starts early) ---
                s_big = spool.tile([P, NT, C_S], bf16, tag="s_big")
                for sq in range(4):
                    tq = NT // 4
                    nc.sync.dma_start(
                        out=s_big[:, sq * tq : (sq + 1) * tq, :],
                        in_=s_pt[:, sq * tq : (sq + 1) * tq, :],
                    )

                # --- LN + matmul into transposed upd table (SBUF-resident) ---
                updT = updp.tile([P, N_RES], f32, tag="updT")
                for i in range(NT):
                    st6 = stats.tile([P, 6], f32)
                    nc.vector.bn_stats(out=st6[:], in_=s_big[:, i, :])
                    mv = stats.tile([P, 2], f32)
                    nc.vector.bn_aggr(out=mv[:], in_=st6[:])
                    std = stats.tile([P, 1], f32)
                    nc.scalar.activation(
                        out=std[:],
                        in_=mv[:, 1:2],
                        func=mybir.ActivationFunctionType.Sqrt,
                        bias=eps_t[:],
                    )
                    rstd = stats.tile([P, 1], f32)
                    nc.vector.reciprocal(out=rstd[:], in_=std[:])
                    t2 = t2p.tile([P, C_S], bf16)
                    nc.vector.tensor_scalar(
                        out=t2[:],
                        in0=s_big[:, i, :],
                        scalar1=mv[:, 0:1],
                        scalar2=rstd[:],
                        op0=mybir.AluOpType.subtract,
                        op1=mybir.AluOpType.mult,
                    )
                    t2t_chunks = []
                    for k in range(KC):
                        tt_ps = psum_tt.tile([P, P], bf16)
                        nc.tensor.transpose(
                            out=tt_ps[:],
                            in_=t2[:, k * P : (k + 1) * P],
                            identity=ident_b[:],
                        )
                        t2t = t2tp.tile([P, P], bf16)
                        nc.vector.tensor_copy(out=t2t[:], in_=tt_ps[:])
                        t2t_chunks.append(t2t)
                    upd_ps = psum_mm.tile([P, P], f32, tag="updps")
                    for k in range(KC):
                        nc.tensor.matmul(
                            upd_ps[:],
                            lhsT=wg_sb[:, k * P : (k + 1) * P],
                            rhs=t2t_chunks[k][:],
                            start=(k == 0),
                            stop=(k == KC - 1),
                        )
                    # PSUM->SBUF move with the beta bias folded in (DVE)
                    nc.vector.tensor_scalar(
                        out=updT[:, i * P : (i + 1) * P],
                        in0=upd_ps[:],
                        scalar1=bias_sb[:],
                        scalar2=None,
                        op0=mybir.AluOpType.add,
                    )

                # --- gather + transpose-back + residual add, 8 chunks ---
                for c in range(NG):
                    at = atoms_pool.tile([P, CT, C_ATOM], bf16, tag="at")
                    nc.sync.dma_start(
                        out=at[:],
                        in_=atom.ap()[c * G : (c + 1) * G, :].rearrange(
                            "(p t) c -> p t c", p=P
                        ),
                    )
                    g = gath_pool.tile([P, G], f32, tag="g")
                    nc.gpsimd.ap_gather(
                        g[:],
                        updT[:],
                        idx_sb[:, c * (G // 16) : (c + 1) * (G // 16)],
                        channels=P,
                        num_elems=N_RES,
                        d=1,
                        num_idxs=G,
                    )
                    ot = out_pool.tile([P, CT, C_ATOM], bf16, tag="ot")
                    for grp in range(CT // GRP):
                        ps4 = psum_g.tile([P, GRP * P], f32)
                        for t2i in range(GRP):
                            t = grp * GRP + t2i
                            sl = ps4[:, t2i * P : (t2i + 1) * P]
                            # transposed gather tile -> PSUM
                            nc.tensor.matmul(
                                sl,
                                lhsT=g[:, t * P : (t + 1) * P],
                                rhs=ident_f[:],
                                is_transpose=True,
                                start=True,
                                stop=False,
                                skip_group_check=True,
                            )
                            # accumulate atom_embed tile on top (identity matmul)
                            nc.tensor.matmul(
                                sl,
                                lhsT=ident_b[:],
                                rhs=at[:, t, :],
                                start=False,
                                stop=True,
                                skip_group_check=True,
                            )
                        nc.scalar.activation(
                            out=ot[:, grp * GRP : (grp + 1) * GRP, :],
                            in_=ps4[:],
                            func=mybir.ActivationFunctionType.Copy,
                        )
                    nc.sync.dma_start(
                        out=out.ap()[c * G : (c + 1) * G, :].rearrange(
                            "(p t) c -> p t c", p=P
                        ),
                        in_=ot[:],
                    )

            if hw_loop is not None:
                with tc.For_i(0, hw_loop, 1):
                    body()
            else:
                for _rep in range(repeat):
                    body()

    nc.compile()
    return nc


def _prep_core_inputs(atom_embed, s, cond_to_s_idx, ln_gamma, ln_beta, W):
    """Host-side sharding + layout marshalling (no math beyond folding the
    LN scale into the weight layout and remapping index values to the SBUF
    table-column layout)."""
    bf16 = ml_dtypes.bfloat16
    # gamma-folded W^T, chunked so cs-chunk k sits at free columns [k*128, ...)
    wg_full = (W * ln_gamma[None, :]).T.astype(np.float32)  # [C_S, C_ATOM]
    wg_host = np.ascontiguousarray(
        wg_full.reshape(KC, P, C_ATOM).transpose(1, 0, 2).reshape(P, C_S)
    ).astype(bf16)
    wt_full = np.ascontiguousarray(W.T.astype(np.float32))  # [C_S, C_ATOM]
    wt_host = np.ascontiguousarray(
        wt_full.reshape(KC, P, C_ATOM).transpose(1, 0, 2).reshape(P, C_S)
    )
    beta_host = np.ascontiguousarray(
        ln_beta.astype(np.float32).reshape(KC, P).T
    )  # [P, KC]

    in_maps = []
    for b in range(B):
        idx = cond_to_s_idx[b].astype(np.int64)
        # s rows are loaded p-major (row r -> partition r//16, tile r%16), so
        # table row r lives at updT column (r%16)*128 + r//16.
        col = (idx % 16) * P + idx // 16
        # gather-list position j of chunk c must hold the column for atom
        # c*G + (j%128)*CT + j//128 so that PE-transposing 128-column groups
        # yields tiles whose partition p = atom (base + p*CT + t); the atom
        # DMA uses the matching p-major layout with 32KB-contiguous lines.
        A = col.reshape(NG, P, CT)  # A[c, p, t]
        L = A.transpose(0, 2, 1).reshape(NG, G)  # L[c, t*128+p]
        # wrap within each chunk: position j -> partition j%16, offset j//16
        Wr = L.reshape(NG, G // 16, 16).transpose(0, 2, 1)  # [NG, 16, G//16]
        blk = np.ascontiguousarray(
            Wr.transpose(1, 0, 2).reshape(16, N_ATOMS // 16)
        )
        idx_rep = np.ascontiguousarray(np.tile(blk, (P // 16, 1))).astype(np.int16)
        in_maps.append(
            {
                "atom": np.ascontiguousarray(atom_embed[b]).astype(bf16),
                "s_in": np.ascontiguousarray(s[b]).astype(bf16),
                "idx16": idx_rep,
                "wg": wg_host,
                "wt": wt_host,
                "beta3": beta_host,
            }
        )
    return in_maps


def kernel(atom_embed, s, cond_to_s_idx, ln_gamma, ln_beta, W):
    global _compiled
    from concourse.bass_utils import run_bass_kernel_spmd

    atom_embed = np.asarray(atom_embed, dtype=np.float32)
    s = np.asarray(s, dtype=np.float32)
    cond_to_s_idx = np.asarray(cond_to_s_idx)
    ln_gamma = np.asarray(ln_gamma, dtype=np.float32)
    ln_beta = np.asarray(ln_beta, dtype=np.float32)
    W = np.asarray(W, dtype=np.float32)

    if _compiled is None:
        _compiled = _build()
    in_maps = _prep_core_inputs(atom_embed, s, cond_to_s_idx, ln_gamma, ln_beta, W)
    res = run_bass_kernel_spmd(_compiled, in_maps, core_ids=list(range(B)))
    out = np.stack(
        [res.results[b]["out"].astype(np.float32) for b in range(B)], axis=0
    )
    return out


# revision 11
# speedup vs baseline: 378.3441x; 2.7953x over previous
"""Bass/Trainium2 kernel for nn_GatherUpdate: LayerNorm + Linear + per-atom
row gather + residual add, data-parallel over batch across 8 NeuronCores.

reference:
    normed = LayerNorm(s) * gamma + beta            # s: [B, 2048, 384]
    upd    = normed @ W.T                           # W: [128, 384] -> [B, 2048, 128]
    out    = atom_embed + upd[:, cond_to_s_idx, :]  # atom_embed: [B, 32768, 128]

Per-core plan (core b handles batch b). Everything bulk moves as bf16
(tolerance is 2e-2, bf16 costs ~5e-3) and every large read rides the SWDGE
dma_gather path, which measures ~5x the bandwidth of plain dma_start here:

  1. s loaded via 2 SWDGE row-gathers (iota indices, 768B rows, p-major).
  2. LN over 16 tiles of [128 res, 384] via bn_stats/bn_aggr; PE-transpose
     normed tiles; matmul against gamma-folded W^T chunks into PSUM; DVE
     folds the (beta @ W.T) bias in during the PSUM->SBUF move (bf16 out).
  3. upd table [2048, 128] bf16 written to DRAM scratch (4 quarter DMAs).
  4. Per 4096-atom chunk: atom_embed rows arrive via 4 iota dma_gathers
     (256B rows), upd rows via 4 indexed dma_gathers (both rotate the 4
     SWDGE queues; the host pre-permutes both int16 index streams so the
     gather output layout matches p-major chunk tiles), one DVE bf16 add,
     one plain dma_start store.
"""

import sys

sys.path.insert(0, "/opt/trn_rl_repo")

import numpy as np
import ml_dtypes

B = 8
N_ATOMS = 32768
N_RES = 2048
C_S = 384
C_ATOM = 128
EPS = 1e-5
P = 128
NT = N_RES // P  # 16 res tiles
KC = C_S // P  # 3 contraction chunks
G = 4096  # atoms per chunk
NG = N_ATOMS // G  # 8 chunks
CT = G // P  # 32 tiles of 128 atoms per chunk
GSUB = 1024  # idxs per dma_gather (SWDGE ring holds ~127 desc/engine)
GQ = G // GSUB  # 4 sub-gathers per chunk
GU = GSUB // P  # 8 rows per partition per sub-gather
NQ = 4  # SWDGE queues

_compiled = None


def _build(repeat=1, hw_loop=None):
    """Build the per-core program. repeat>1 python-unrolls the body N times;
    hw_loop=N wraps the body in a device-side For_i hardware loop (used for
    low-noise timing: signal scales with N at constant program size)."""
    import concourse.bacc as bacc
    import concourse.tile as tile
    from concourse import mybir
    from concourse.masks import make_identity
    from concourse.tile import add_dep_helper

    f32 = mybir.dt.float32
    bf16 = mybir.dt.bfloat16
    i16 = mybir.dt.int16

    nc = bacc.Bacc(
        "TRN2", target_bir_lowering=False, debug=False, num_swdge_queues=NQ
    )

    atom = nc.dram_tensor("atom", [N_ATOMS, C_ATOM], bf16, kind="ExternalInput")
    s_in = nc.dram_tensor("s_in", [N_RES, C_S], bf16, kind="ExternalInput")
    gidx = nc.dram_tensor("gidx", [P, N_ATOMS // 16], i16, kind="ExternalInput")
    aidx = nc.dram_tensor("aidx", [P, N_ATOMS // 16], i16, kind="ExternalInput")
    sidx = nc.dram_tensor("sidx", [P, N_RES // 16], i16, kind="ExternalInput")
    wg = nc.dram_tensor("wg", [P, C_S], bf16, kind="ExternalInput")
    wt = nc.dram_tensor("wt", [P, C_S], f32, kind="ExternalInput")
    beta3 = nc.dram_tensor("beta3", [P, KC], f32, kind="ExternalInput")
    out = nc.dram_tensor("out", [N_ATOMS, C_ATOM], bf16, kind="ExternalOutput")
    upd_dram = nc.dram_tensor("upd_scratch", [N_RES, C_ATOM], bf16, kind="Internal")

    with tile.TileContext(nc) as tc:
        with (
            tc.tile_pool(name="singles", bufs=1) as singles,
            tc.tile_pool(name="spool", bufs=2) as spool,
            tc.tile_pool(name="updp", bufs=2) as updp,
            tc.tile_pool(name="stats", bufs=4) as stats,
            tc.tile_pool(name="t2p", bufs=3) as t2p,
            tc.tile_pool(name="t2tp", bufs=6) as t2tp,
            tc.tile_pool(name="psum_tt", bufs=3, space="PSUM") as psum_tt,
            tc.tile_pool(name="psum_mm", bufs=3, space="PSUM") as psum_mm,
            tc.tile_pool(name="atoms", bufs=3) as atoms_pool,
            tc.tile_pool(name="gath", bufs=3) as gath_pool,
        ):
            # --- constants / small inputs (loaded once) ---
            gidx_sb = singles.tile([P, N_ATOMS // 16], i16)
            nc.sync.dma_start(out=gidx_sb[:], in_=gidx.ap())
            aidx_sb = singles.tile([P, N_ATOMS // 16], i16)
            nc.sync.dma_start(out=aidx_sb[:], in_=aidx.ap())
            sidx_sb = singles.tile([P, N_RES // 16], i16)
            nc.sync.dma_start(out=sidx_sb[:], in_=sidx.ap())
            wg_sb = singles.tile([P, C_S], bf16)
            nc.sync.dma_start(out=wg_sb[:], in_=wg.ap())
            wt_sb = singles.tile([P, C_S], f32)
            nc.sync.dma_start(out=wt_sb[:], in_=wt.ap())
            beta_sb = singles.tile([P, KC], f32)
            nc.sync.dma_start(out=beta_sb[:], in_=beta3.ap())

            ident_b = singles.tile([P, P], bf16)
            make_identity(nc, ident_b[:])
            eps_t = singles.tile([P, 1], f32)
            nc.vector.memset(eps_t[:], EPS)
            ones1 = singles.tile([1, P], f32)
            nc.vector.memset(ones1[:], 1.0)

            # --- bias row (beta @ W.T), broadcast to all partitions ---
            bias_ps = psum_mm.tile([P, C_ATOM], f32, tag="updps")
            for k in range(KC):
                nc.tensor.matmul(
                    bias_ps[0:1, :],
                    lhsT=beta_sb[:, k : k + 1],
                    rhs=wt_sb[:, k * P : (k + 1) * P],
                    start=(k == 0),
                    stop=(k == KC - 1),
                )
            bias_row = singles.tile([1, C_ATOM], f32)
            nc.vector.tensor_copy(out=bias_row[:], in_=bias_ps[0:1, :])
            bias_bc_ps = psum_mm.tile([P, C_ATOM], f32, tag="updps")
            nc.tensor.matmul(
                bias_bc_ps[:], lhsT=ones1[:], rhs=bias_row[:], start=True, stop=True
            )
            bias_bc = singles.tile([P, C_ATOM], f32)
            nc.vector.tensor_copy(out=bias_bc[:], in_=bias_bc_ps[:])

            prev_gathers = []
            # SWDGE sem lanes are handed out round-robin (mod 8) per Pool-DMA
            # instruction and each lane is locked to one queue, so queue_num
            # must track the Pool-DMA ordinal mod NQ to stay consistent.
            swq = [0]

            def next_q():
                v = swq[0] % NQ
                swq[0] += 1
                return v

            def body():
                nonlocal prev_gathers
                # --- s load: 2 SWDGE row-gathers (iota idx, p-major rows) ---
                s_big = spool.tile([P, NT, C_S], bf16, tag="s_big")
                for q in range(2):
                    nc.gpsimd.dma_gather(
                        s_big[:, q * (NT // 2) : (q + 1) * (NT // 2), :],
                        s_in.ap(),
                        sidx_sb[:, q * (GSUB // 16) : (q + 1) * (GSUB // 16)],
                        GSUB,
                        GSUB,
                        C_S,
                        queue_num=next_q(),
                    )

                # --- LN + matmul into upd table tiles ---
                upd_big = updp.tile([P, NT, C_ATOM], bf16, tag="upd_big")
                for i in range(NT):
                    st6 = stats.tile([P, 6], f32)
                    nc.vector.bn_stats(out=st6[:], in_=s_big[:, i, :])
                    mv = stats.tile([P, 2], f32)
                    nc.vector.bn_aggr(out=mv[:], in_=st6[:])
                    std = stats.tile([P, 1], f32)
                    nc.scalar.activation(
                        out=std[:],
                        in_=mv[:, 1:2],
                        func=mybir.ActivationFunctionType.Sqrt,
                        bias=eps_t[:],
                    )
                    rstd = stats.tile([P, 1], f32)
                    nc.vector.reciprocal(out=rstd[:], in_=std[:])
                    t2 = t2p.tile([P, C_S], bf16)
                    nc.vector.tensor_scalar(
                        out=t2[:],
                        in0=s_big[:, i, :],
                        scalar1=mv[:, 0:1],
                        scalar2=rstd[:],
                        op0=mybir.AluOpType.subtract,
                        op1=mybir.AluOpType.mult,
                    )
                    t2t_chunks = []
                    for k in range(KC):
                        tt_ps = psum_tt.tile([P, P], bf16)
                        nc.tensor.transpose(
                            out=tt_ps[:],
                            in_=t2[:, k * P : (k + 1) * P],
                            identity=ident_b[:],
                        )
                        t2t = t2tp.tile([P, P], bf16)
                        nc.vector.tensor_copy(out=t2t[:], in_=tt_ps[:])
                        t2t_chunks.append(t2t)
                    upd_ps = psum_mm.tile([P, C_ATOM], f32, tag="updps")
                    for k in range(KC):
                        nc.tensor.matmul(
                            upd_ps[:],
                            lhsT=t2t_chunks[k][:],
                            rhs=wg_sb[:, k * P : (k + 1) * P],
                            start=(k == 0),
                            stop=(k == KC - 1),
                        )
                    # PSUM->SBUF move (bf16) with the beta bias folded in
                    nc.vector.tensor_tensor(
                        out=upd_big[:, i, :],
                        in0=upd_ps[:],
                        in1=bias_bc[:],
                        op=mybir.AluOpType.add,
                    )

                # --- upd table to DRAM (p-major rows: row p*16+i) ---
                upd_stores = []
                upd_pt = upd_dram.ap().rearrange("(p t) c -> p t c", p=P)
                for sq in range(4):
                    tq = NT // 4
                    us = nc.sync.dma_start(
                        out=upd_pt[:, sq * tq : (sq + 1) * tq, :],
                        in_=upd_big[:, sq * tq : (sq + 1) * tq, :],
                    )
                    upd_stores.append(us)
                # WAR: don't overwrite the table while last rep still gathers
                for pg in prev_gathers:
                    for us in upd_stores:
                        add_dep_helper(
                            us.ins, pg.ins, reason="WAR on upd table across reps"
                        )
                prev_gathers = []

                # --- per chunk: iota-gather atoms, idx-gather upd, add, store ---
                for c in range(NG):
                    at = atoms_pool.tile([P, CT, C_ATOM], bf16, tag="at")
                    g = gath_pool.tile([P, CT, C_ATOM], bf16, tag="g")
                    for q in range(GQ):
                        gc = c * GQ + q
                        nc.gpsimd.dma_gather(
                            at[:, q * GU : (q + 1) * GU, :],
                            atom.ap(),
                            aidx_sb[
                                :, gc * (GSUB // 16) : (gc + 1) * (GSUB // 16)
                            ],
                            GSUB,
                            GSUB,
                            C_ATOM,
                            queue_num=next_q(),
                        )
                        gi = nc.gpsimd.dma_gather(
                            g[:, q * GU : (q + 1) * GU, :],
                            upd_dram.ap(),
                            gidx_sb[
                                :, gc * (GSUB // 16) : (gc + 1) * (GSUB // 16)
                            ],
                            GSUB,
                            GSUB,
                            C_ATOM,
                            queue_num=next_q(),
                        )
                        for us in upd_stores:
                            add_dep_helper(
                                gi.ins, us.ins, reason="upd table must be in DRAM"
                            )
                        prev_gathers.append(gi)
                    nc.vector.tensor_tensor(
                        out=at[:], in0=at[:], in1=g[:], op=mybir.AluOpType.add
                    )
                    nc.sync.dma_start(
                        out=out.ap()[c * G : (c + 1) * G, :].rearrange(
                            "(p t) c -> p t c", p=P
                        ),
                        in_=at[:],
                    )

            if hw_loop is not None:
                with tc.For_i(0, hw_loop, 1):
                    body()
            else:
                for _rep in range(repeat):
                    body()

    nc.compile()
    return nc


def _wrap16(vals):
    """[nchunks, 128, u] gather values -> [128, nchunks*u*128//16] int16 in
    the SWDGE index layout: list position j of chunk k sits at partition
    (j%16), free offset k*(G16) + j//16, replicated across 16-partition
    groups; position j = u*128 + p feeds output slot [p, u]."""
    nch, p128, u = vals.shape
    gsz = p128 * u
    L = vals.transpose(0, 2, 1).reshape(nch, gsz)  # j = u*128 + p
    Wr = L.reshape(nch, gsz // 16, 16).transpose(0, 2, 1)  # [nch, 16, gsz//16]
    blk = np.ascontiguousarray(Wr.transpose(1, 0, 2).reshape(16, nch * gsz // 16))
    return np.ascontiguousarray(np.tile(blk, (8, 1))).astype(np.int16)


def _prep_core_inputs(atom_embed, s, cond_to_s_idx, ln_gamma, ln_beta, W):
    """Host-side sharding + layout marshalling (no math beyond folding the
    LN scale into the weight layout and permuting index streams)."""
    bf16 = ml_dtypes.bfloat16
    # gamma-folded W^T, chunked so cs-chunk k sits at free columns [k*128, ...)
    wg_full = (W * ln_gamma[None, :]).T.astype(np.float32)  # [C_S, C_ATOM]
    wg_host = np.ascontiguousarray(
        wg_full.reshape(KC, P, C_ATOM).transpose(1, 0, 2).reshape(P, C_S)
    ).astype(bf16)
    wt_full = np.ascontiguousarray(W.T.astype(np.float32))
    wt_host = np.ascontiguousarray(
        wt_full.reshape(KC, P, C_ATOM).transpose(1, 0, 2).reshape(P, C_S)
    )
    beta_host = np.ascontiguousarray(
        ln_beta.astype(np.float32).reshape(KC, P).T
    )  # [P, KC]

    # atom iota gather: sub-gather gc=(c,q), position j=u*128+p -> atom row
    # c*G + p*CT + q*GU + u (p-major chunk tiles with 8KB partition lines)
    ar = np.arange(N_ATOMS, dtype=np.int64)
    aidx_host = _wrap16(
        ar.reshape(NG, P, GQ, GU).transpose(0, 2, 1, 3).reshape(NG * GQ, P, GU)
    )
    # s iota gather: position j of sub-gather q -> s row p*16 + q*8 + u
    sr = np.arange(N_RES, dtype=np.int64)  # row p*16 + i at [p, i]
    sidx_host = _wrap16(
        sr.reshape(P, 2, NT // 2).transpose(1, 0, 2).reshape(2, P, NT // 2)
    )

    in_maps = []
    for b in range(B):
        idx = cond_to_s_idx[b].astype(np.int64)
        gidx_host = _wrap16(
            idx.reshape(NG, P, GQ, GU).transpose(0, 2, 1, 3).reshape(NG * GQ, P, GU)
        )
        in_maps.append(
            {
                "atom": np.ascontiguousarray(atom_embed[b]).astype(bf16),
                "s_in": np.ascontiguousarray(s[b]).astype(bf16),
                "gidx": gidx_host,
                "aidx": aidx_host,
                "sidx": sidx_host,
                "wg": wg_host,
                "wt": wt_host,
                "beta3": beta_host,
            }
        )
    return in_maps


def kernel(atom_embed, s, cond_to_s_idx, ln_gamma, ln_beta, W):
    global _compiled
    from concourse.bass_utils import run_bass_kernel_spmd

    atom_embed = np.asarray(atom_embed, dtype=np.float32)
    s = np.asarray(s, dtype=np.float32)
    cond_to_s_idx = np.asarray(cond_to_s_idx)
    ln_gamma = np.asarray(ln_gamma, dtype=np.float32)
    ln_beta = np.asarray(ln_beta, dtype=np.float32)
    W = np.asarray(W, dtype=np.float32)

    if _compiled is None:
        _compiled = _build()
    in_maps = _prep_core_inputs(atom_embed, s, cond_to_s_idx, ln_gamma, ln_beta, W)
    res = run_bass_kernel_spmd(_compiled, in_maps, core_ids=list(range(B)))
    out = np.stack(
        [res.results[b]["out"].astype(np.float32) for b in range(B)], axis=0
    )
    return out
